# revision 7
# baseline (speedup 1.0000x reference)
"""AutomatonPELayer kernel for 8 Trainium2 NeuronCores.

Math: pe[j] = T^j @ x0 (j = 0..L-1), out = pe @ W.T + b, with T orthogonal
[128,128], L = 131072, embed dim 512, fp32.

Strategy (sequence-sharded, K-decimated row layout):
- Core m owns rows [m*16384, (m+1)*16384). Within a core, the rows are
  produced in NTILES=16 store tiles of 1024 rows. Tile t's SBUF layout
  puts rows base+8p+j (j=0..7) in partition p, so the 2 MB store is 16 KB
  of *contiguous* DRAM per partition (vs 2 KB/row with the naive block
  layout) -> near-fabric-rate store descriptors.
- Row base+8p+j = (T^(base+8p) x0)^T (W T^j)^T: one anchor matrix A_t
  (columns T^(base+8p) x0, host-precomputed in f64) is the stationary
  operand for 8 matmuls against Wj = (T^j)^T W^T (j = 0..7, replicated).
  PSUM pairs (j=2q, 2q+1) share a 2-bank [128,1024] psum tile, drained by
  a single DVE/ACT copy, 4 copies per tile.
- The PE array is clock-gated to 1.2 GHz until ~3.4 us of sustained
  activity (HAM). A chain of dummy matmuls during the input-load window
  flips it to 2.4 GHz before the real matmuls start.
- First-needed inputs (A_0, all four Wj pairs) ride the two fast HWDGE
  queues; the remaining anchors ride the gpsimd SWDGE queue in parallel
  with the store stream. Tile 0 is stored in four 512 KB sub-stores so
  the HBM write stream starts as soon as the first copy lands.
- b is folded in on the host only if nonzero (it is zero in this
  problem's setup_inputs); the device path is a pure GEMM.
"""

import sys

if "/opt/trn_rl_repo" not in sys.path:
    sys.path.insert(0, "/opt/trn_rl_repo")

import numpy as np

L = 131072
S = 128  # num states (= partition dim = contraction dim)
E = 512  # embed dim
NCORES = 8
CHUNK = L // NCORES  # 16384 rows per core
K = 8  # rows per partition per store tile (decimation factor)
TILE_ROWS = S * K  # 1024 rows per store tile
NTILES = CHUNK // TILE_ROWS  # 16 store tiles per core
NPAIR = K // 2  # 4 psum pairs per tile
WARMUP = 14  # dummy matmuls to trip the HAM clock gate

_prog_cache = {}


def _split_multi_waits(nc, mybir):
    """This walrus build accepts only ONE sync-wait per instruction
    (setupSyncWait: 'Too many sync wait commands'). Tile attaches the
    full wait list to the consuming instruction; hoist all but the
    last wait onto single-wait NoOps placed immediately before it on
    the same engine, preserving per-engine program order."""
    uid = 0
    for fn in nc.m.functions:
        for bb in fn.blocks:
            new = []
            changed = False
            for inst in bb.instructions:
                si = inst.sync_info
                waits = list(si.on_wait) if si is not None else []
                if len(waits) > 1:
                    changed = True
                    for w in waits[:-1]:
                        nop = mybir.InstNoOp(
                            name=f"splitw_{uid}",
                            engine=inst.engine,
                            sync_info=mybir.SyncInfo(on_wait=[w], on_update=[]),
                            bass_nofuse=True,
                        )
                        uid += 1
                        new.append(nop)
                    si.on_wait = [waits[-1]]
                new.append(inst)
            if changed:
                bb.instructions = new


def _build_program():
    if "nc" in _prog_cache:
        return _prog_cache["nc"]

    import concourse.bass as bass
    import concourse.tile as tile
    from concourse import mybir

    f32 = mybir.dt.float32
    f16 = mybir.dt.float16
    nc = bass.Bass("TRN2", target_bir_lowering=False, debug=False, num_devices=NCORES)

    # anchors differ per core; wpairs replicated. fp16 operands: single-pass
    # PE matmul (1 cycle/column) with fast weight load; fp32 PSUM accumulate.
    anchors = nc.dram_tensor("anchors", [NTILES, S, S], f16, kind="ExternalInput").ap()
    wpairs = nc.dram_tensor("wpairs", [NPAIR, S, 2 * E], f16, kind="ExternalInput").ap()
    out = nc.dram_tensor("out", [CHUNK, E], f32, kind="ExternalOutput").ap()

    anchors_v = anchors.rearrange("t s p -> s t p")  # [S, NTILES, S]
    wpairs_v = wpairs.rearrange("q s e -> s q e")  # [S, NPAIR, 2E]

    # Row base+8p+j of the output lives at DRAM row t*1024 + p*8 + j:
    # full-tile view [t, p, (k e)] is 16 KB contiguous per partition.
    out_t = out.rearrange("(t p k) e -> t p (k e)", p=S, k=K)  # [NTILES, S, K*E]
    # Sub-store view per psum pair q (rows j=2q, 2q+1): 4 KB per partition.
    out_q = out.rearrange("(t p q j) e -> t q p (j e)", p=S, q=NPAIR, j=2)

    with tile.TileContext(nc) as tc:
        with (
            tc.tile_pool(name="singles", bufs=1) as singles,
            tc.tile_pool(name="opool", bufs=6) as opool,
            tc.tile_pool(name="psum", bufs=3, space="PSUM") as psum,
            tc.tile_pool(name="warm", bufs=1, space="PSUM") as warm,
        ):
            anch_t = singles.tile([S, NTILES, S], f16)
            wp_t = singles.tile([S, NPAIR, 2 * E], f16)
            dummy = singles.tile([S, S], f16)

            # First-needed inputs on the two HWDGE rings, in need order:
            # sync ring: W(0|1), W(4|5); scalar ring: A_0, W(2|3), A_1, W(6|7).
            nc.sync.dma_start(out=wp_t[:, 0, :], in_=wpairs_v[:, 0, :])
            nc.scalar.dma_start(out=anch_t[:, 0, :], in_=anchors_v[:, 0, :])
            nc.scalar.dma_start(out=wp_t[:, 1, :], in_=wpairs_v[:, 1, :])
            nc.sync.dma_start(out=wp_t[:, 2, :], in_=wpairs_v[:, 2, :])
            nc.scalar.dma_start(out=anch_t[:, 1, :], in_=anchors_v[:, 1, :])
            nc.scalar.dma_start(out=wp_t[:, 3, :], in_=wpairs_v[:, 3, :])
            # Remaining anchors on the gpsimd SWDGE queue, off the store rings.
            for t in range(2, 5):
                nc.gpsimd.dma_start(out=anch_t[:, t, :], in_=anchors_v[:, t, :])
            nc.gpsimd.dma_start(out=anch_t[:, 5:, :], in_=anchors_v[:, 5:, :])

            # PE warmup: sustained dummy matmuls (no input deps) flip the
            # HAM clock gate to 2.4 GHz while the inputs load. gpsimd is
            # the earliest-released engine, so it writes the dummy.
            nc.gpsimd.memset(dummy, 0.0)
            for _ in range(WARMUP):
                pw = warm.tile([S, S], f32)
                nc.tensor.matmul(pw, dummy, dummy, start=True, stop=True)

            for t in range(NTILES):
                o_t = opool.tile([S, K * E], f32)
                for q in range(NPAIR):
                    ps = psum.tile([S, 2 * E], f32)
                    for j in range(2):
                        nc.tensor.matmul(
                            ps[:, j * E : (j + 1) * E],
                            anch_t[:, t, :],
                            wp_t[:, q, j * E : (j + 1) * E],
                            start=True,
                            stop=True,
                        )
                    o_sl = o_t[:, q * 2 * E : (q + 1) * 2 * E]
                    if q % 2 == 0:
                        nc.vector.tensor_copy(o_sl, ps)
                    else:
                        nc.scalar.copy(out=o_sl, in_=ps)
                    if t == 0:
                        # Head: stream each 512 KB quarter as soon as its
                        # copy lands.
                        if q % 2 == 0:
                            nc.sync.dma_start(out=out_q[0, q], in_=o_sl)
                        else:
                            nc.scalar.dma_start(out=out_q[0, q], in_=o_sl)
                if t == NTILES - 1:
                    # Tail: drain the last tile as two 1 MB halves on both
                    # rings so the final store finishes sooner.
                    half = K * E // 2
                    nc.sync.dma_start(
                        out=out_t[t][:, :half], in_=o_t[:, :half]
                    )
                    nc.scalar.dma_start(
                        out=out_t[t][:, half:], in_=o_t[:, half:]
                    )
                elif t > 0:
                    if t % 2 == 0:
                        nc.sync.dma_start(out=out_t[t], in_=o_t)
                    else:
                        nc.scalar.dma_start(out=out_t[t], in_=o_t)

    _split_multi_waits(nc, mybir)
    _prog_cache["nc"] = nc
    return nc


def _host_precompute(pos_initial, pos_transition, W):
    """float64 host prep: decimated per-core anchors + T^j-folded weights."""
    T = np.asarray(pos_transition, np.float64)
    x0 = np.asarray(pos_initial, np.float64).reshape(S)
    W64 = np.asarray(W, np.float64)

    # T^8 and T^1024 by repeated squaring
    T8 = T.copy()
    for _ in range(3):
        T8 = T8 @ T8
    T1024 = T8.copy()
    for _ in range(7):
        T1024 = T1024 @ T1024

    # Wj = (T^j)^T @ W^T for j = 0..K-1, paired as [NPAIR, S, 2E]
    Tj = np.eye(S)
    wjs = []
    for j in range(K):
        wjs.append(np.ascontiguousarray(Tj.T @ W64.T))
        Tj = T @ Tj
    wpairs = np.stack(
        [np.concatenate([wjs[2 * q], wjs[2 * q + 1]], axis=1) for q in range(NPAIR)]
    ).astype(np.float16)

    # Anchor columns: y_i = T^(8 i) x0. A(m, t)[:, p] = y[(m*16 + t)*128 + p].
    C = np.empty((S, S), np.float64)
    v = x0.copy()
    C[:, 0] = v
    for i in range(1, S):
        v = T8 @ v
        C[:, i] = v
    anchor_steps = []
    A = C
    for _ in range(NCORES * NTILES):
        anchor_steps.append(A)
        A = T1024 @ A
    anchors_all = np.asarray(anchor_steps, np.float64).reshape(NCORES, NTILES, S, S)
    anchors = [
        np.ascontiguousarray(anchors_all[m]).astype(np.float16) for m in range(NCORES)
    ]
    return anchors, wpairs


def kernel(sentence_len, pos_initial, pos_transition, W, b):
    from concourse.bass_utils import run_bass_kernel_spmd

    assert int(sentence_len) == L, f"kernel hardcodes L={L}, got {sentence_len}"
    b = np.asarray(b, np.float32)

    anchors, wpairs = _host_precompute(pos_initial, pos_transition, W)

    nc = _build_program()
    in_maps = [{"anchors": anchors[m], "wpairs": wpairs} for m in range(NCORES)]
    res = run_bass_kernel_spmd(nc, in_maps, core_ids=list(range(NCORES)))
    full = np.concatenate([res.results[m]["out"] for m in range(NCORES)], axis=0)
    if np.any(b != 0):
        full = full + b[None, :]
    return full


# revision 13
# speedup vs baseline: 1.0623x; 1.0623x over previous
"""AutomatonPELayer kernel for 8 Trainium2 NeuronCores.

Math: pe[j] = T^j @ x0 (j = 0..L-1), out = pe @ W.T + b, with T orthogonal
[128,128], L = 131072, embed dim 512, fp32.

Strategy (sequence-sharded, K-decimated row layout):
- Core m owns rows [m*16384, (m+1)*16384). Within a core, the rows are
  produced in NTILES=16 store tiles of 1024 rows. Tile t's SBUF layout
  puts rows base+8p+j (j=0..7) in partition p, so the 2 MB store is 16 KB
  of *contiguous* DRAM per partition (vs 2 KB/row with the naive block
  layout) -> near-fabric-rate store descriptors.
- Row base+8p+j = (T^(base+8p) x0)^T (W T^j)^T: one anchor matrix A_t
  (columns T^(base+8p) x0, host-precomputed in f64) is the stationary
  operand for 8 matmuls against Wj = (T^j)^T W^T (j = 0..7, replicated).
  PSUM pairs (j=2q, 2q+1) share a 2-bank [128,1024] psum tile, drained by
  a single DVE/ACT copy, 4 copies per tile.
- The PE array is clock-gated to 1.2 GHz until ~3.4 us of sustained
  activity (HAM). A chain of dummy matmuls during the input-load window
  flips it to 2.4 GHz before the real matmuls start.
- First-needed inputs (A_0, all four Wj pairs) ride the two fast HWDGE
  queues; the remaining anchors ride the gpsimd SWDGE queue in parallel
  with the store stream. Tile 0 is stored in four 512 KB sub-stores so
  the HBM write stream starts as soon as the first copy lands.
- b is folded in on the host only if nonzero (it is zero in this
  problem's setup_inputs); the device path is a pure GEMM.
"""

import sys

if "/opt/trn_rl_repo" not in sys.path:
    sys.path.insert(0, "/opt/trn_rl_repo")

import numpy as np

L = 131072
S = 128  # num states (= partition dim = contraction dim)
E = 512  # embed dim
NCORES = 8
CHUNK = L // NCORES  # 16384 rows per core
K = 8  # rows per partition per store tile (decimation factor)
TILE_ROWS = S * K  # 1024 rows per store tile
NTILES = CHUNK // TILE_ROWS  # 16 store tiles per core
NPAIR = K // 2  # 4 psum pairs per tile
WARMUP = 20  # dummy matmuls to trip the HAM clock gate

_prog_cache = {}


def _split_multi_waits(nc, mybir):
    """This walrus build accepts only ONE sync-wait per instruction
    (setupSyncWait: 'Too many sync wait commands'). Tile attaches the
    full wait list to the consuming instruction; hoist all but the
    last wait onto single-wait NoOps placed immediately before it on
    the same engine, preserving per-engine program order."""
    uid = 0
    for fn in nc.m.functions:
        for bb in fn.blocks:
            new = []
            changed = False
            for inst in bb.instructions:
                si = inst.sync_info
                waits = list(si.on_wait) if si is not None else []
                if len(waits) > 1:
                    changed = True
                    for w in waits[:-1]:
                        nop = mybir.InstNoOp(
                            name=f"splitw_{uid}",
                            engine=inst.engine,
                            sync_info=mybir.SyncInfo(on_wait=[w], on_update=[]),
                            bass_nofuse=True,
                        )
                        uid += 1
                        new.append(nop)
                    si.on_wait = [waits[-1]]
                new.append(inst)
            if changed:
                bb.instructions = new


def _build_program():
    if "nc" in _prog_cache:
        return _prog_cache["nc"]

    import concourse.bass as bass
    import concourse.tile as tile
    from concourse import mybir

    f32 = mybir.dt.float32
    f16 = mybir.dt.float16
    nc = bass.Bass("TRN2", target_bir_lowering=False, debug=False, num_devices=NCORES)

    # anchors differ per core; wpairs replicated. fp16 operands: single-pass
    # PE matmul (1 cycle/column) with fast weight load; fp32 PSUM accumulate.
    anchors = nc.dram_tensor("anchors", [NTILES, S, S], f16, kind="ExternalInput").ap()
    wpairs = nc.dram_tensor("wpairs", [NPAIR, S, 2 * E], f16, kind="ExternalInput").ap()
    out = nc.dram_tensor("out", [CHUNK, E], f32, kind="ExternalOutput").ap()

    anchors_v = anchors.rearrange("t s p -> s t p")  # [S, NTILES, S]
    wpairs_v = wpairs.rearrange("q s e -> s q e")  # [S, NPAIR, 2E]

    # Row base+8p+j of the output lives at DRAM row t*1024 + p*8 + j:
    # full-tile view [t, p, (k e)] is 16 KB contiguous per partition.
    out_t = out.rearrange("(t p k) e -> t p (k e)", p=S, k=K)  # [NTILES, S, K*E]
    # Sub-store view per psum pair q (rows j=2q, 2q+1): 4 KB per partition.
    out_q = out.rearrange("(t p q j) e -> t q p (j e)", p=S, q=NPAIR, j=2)

    with tile.TileContext(nc) as tc:
        with (
            tc.tile_pool(name="singles", bufs=1) as singles,
            tc.tile_pool(name="opool", bufs=5) as opool,
            tc.tile_pool(name="psum", bufs=4, space="PSUM") as psum,
        ):
            anch_t = singles.tile([S, NTILES, S], f16)
            wp_t = singles.tile([S, NPAIR, 2 * E], f16)
            dummy = singles.tile([S, S], f16)

            # PE warmup: sustained dummy matmuls (no input deps) flip the
            # HAM clock gate to 2.4 GHz while the inputs load.
            nc.vector.memset(dummy, 0.0)

            # First-needed inputs on the two HWDGE rings, in need order:
            # sync ring: W(0|1), W(4|5); scalar ring: A_0, W(2|3), A_1, W(6|7).
            nc.sync.dma_start(out=wp_t[:, 0, :], in_=wpairs_v[:, 0, :])
            nc.scalar.dma_start(out=anch_t[:, 0, :], in_=anchors_v[:, 0, :])
            nc.scalar.dma_start(out=wp_t[:, 1, :], in_=wpairs_v[:, 1, :])
            nc.sync.dma_start(out=wp_t[:, 2, :], in_=wpairs_v[:, 2, :])
            nc.scalar.dma_start(out=anch_t[:, 1, :], in_=anchors_v[:, 1, :])
            nc.scalar.dma_start(out=wp_t[:, 3, :], in_=wpairs_v[:, 3, :])
            # Remaining anchors individually on the gpsimd SWDGE queue, off
            # the store rings; per-anchor semaphores so tile t waits only
            # for its own anchor, not a whole slab.
            for t in range(2, NTILES):
                nc.gpsimd.dma_start(out=anch_t[:, t, :], in_=anchors_v[:, t, :])

            for _ in range(WARMUP):
                pw = psum.tile([S, 2 * E], f32, tag="pp")
                nc.tensor.matmul(pw[:, :S], dummy, dummy, start=True, stop=True)

            for t in range(NTILES):
                o_t = opool.tile([S, K * E], f32)
                for q in range(NPAIR):
                    ps = psum.tile([S, 2 * E], f32, tag="pp")
                    for j in range(2):
                        nc.tensor.matmul(
                            ps[:, j * E : (j + 1) * E],
                            anch_t[:, t, :],
                            wp_t[:, q, j * E : (j + 1) * E],
                            start=True,
                            stop=True,
                        )
                    o_sl = o_t[:, q * 2 * E : (q + 1) * 2 * E]
                    if q % 2 == 0:
                        nc.vector.tensor_copy(o_sl, ps)
                    else:
                        nc.scalar.copy(out=o_sl, in_=ps)
                    if t == 0:
                        # Head: stream each 512 KB quarter as soon as its
                        # copy lands.
                        if q % 2 == 0:
                            nc.sync.dma_start(out=out_q[0, q], in_=o_sl)
                        else:
                            nc.scalar.dma_start(out=out_q[0, q], in_=o_sl)
                if t == NTILES - 1:
                    # Tail: drain the last tile as two 1 MB halves on both
                    # rings so the final store finishes sooner.
                    half = K * E // 2
                    nc.sync.dma_start(
                        out=out_t[t][:, :half], in_=o_t[:, :half]
                    )
                    nc.scalar.dma_start(
                        out=out_t[t][:, half:], in_=o_t[:, half:]
                    )
                elif t > 0:
                    if t % 2 == 0:
                        nc.sync.dma_start(out=out_t[t], in_=o_t)
                    else:
                        nc.scalar.dma_start(out=out_t[t], in_=o_t)

    _split_multi_waits(nc, mybir)
    _prog_cache["nc"] = nc
    return nc


def _host_precompute(pos_initial, pos_transition, W):
    """float64 host prep: decimated per-core anchors + T^j-folded weights."""
    T = np.asarray(pos_transition, np.float64)
    x0 = np.asarray(pos_initial, np.float64).reshape(S)
    W64 = np.asarray(W, np.float64)

    # T^8 and T^1024 by repeated squaring
    T8 = T.copy()
    for _ in range(3):
        T8 = T8 @ T8
    T1024 = T8.copy()
    for _ in range(7):
        T1024 = T1024 @ T1024

    # Wj = (T^j)^T @ W^T for j = 0..K-1, paired as [NPAIR, S, 2E]
    Tj = np.eye(S)
    wjs = []
    for j in range(K):
        wjs.append(np.ascontiguousarray(Tj.T @ W64.T))
        Tj = T @ Tj
    wpairs = np.stack(
        [np.concatenate([wjs[2 * q], wjs[2 * q + 1]], axis=1) for q in range(NPAIR)]
    ).astype(np.float16)

    # Anchor columns: y_i = T^(8 i) x0. A(m, t)[:, p] = y[(m*16 + t)*128 + p].
    C = np.empty((S, S), np.float64)
    v = x0.copy()
    C[:, 0] = v
    for i in range(1, S):
        v = T8 @ v
        C[:, i] = v
    anchor_steps = []
    A = C
    for _ in range(NCORES * NTILES):
        anchor_steps.append(A)
        A = T1024 @ A
    anchors_all = np.asarray(anchor_steps, np.float64).reshape(NCORES, NTILES, S, S)
    anchors = [
        np.ascontiguousarray(anchors_all[m]).astype(np.float16) for m in range(NCORES)
    ]
    return anchors, wpairs


def kernel(sentence_len, pos_initial, pos_transition, W, b):
    from concourse.bass_utils import run_bass_kernel_spmd

    assert int(sentence_len) == L, f"kernel hardcodes L={L}, got {sentence_len}"
    b = np.asarray(b, np.float32)

    anchors, wpairs = _host_precompute(pos_initial, pos_transition, W)

    nc = _build_program()
    in_maps = [{"anchors": anchors[m], "wpairs": wpairs} for m in range(NCORES)]
    res = run_bass_kernel_spmd(nc, in_maps, core_ids=list(range(NCORES)))
    full = np.concatenate([res.results[m]["out"] for m in range(NCORES)], axis=0)
    if np.any(b != 0):
        full = full + b[None, :]
    return full


# revision 16
# speedup vs baseline: 1.1805x; 1.1113x over previous
"""AutomatonPELayer kernel for 8 Trainium2 NeuronCores.

Math: pe[j] = T^j @ x0 (j = 0..L-1), out = pe @ W.T + b, with T orthogonal
[128,128], L = 131072, embed dim 512, fp32.

Strategy (sequence-sharded, K-decimated row layout):
- Core m owns rows [m*16384, (m+1)*16384). Within a core, the rows are
  produced in NTILES=16 store tiles of 1024 rows. Tile t's SBUF layout
  puts rows base+8p+j (j=0..7) in partition p, so the 2 MB store is 16 KB
  of *contiguous* DRAM per partition (vs 2 KB/row with the naive block
  layout) -> near-fabric-rate store descriptors.
- Row base+8p+j = (T^(base+8p) x0)^T (W T^j)^T: one anchor matrix A_t
  (columns T^(base+8p) x0, host-precomputed in f64) is the stationary
  operand for 8 matmuls against Wj = (T^j)^T W^T (j = 0..7, replicated).
  PSUM pairs (j=2q, 2q+1) share a 2-bank [128,1024] psum tile, drained by
  a single DVE/ACT copy, 4 copies per tile.
- The PE array is clock-gated to 1.2 GHz until ~3.4 us of sustained
  activity (HAM). A chain of dummy matmuls during the input-load window
  flips it to 2.4 GHz before the real matmuls start.
- First-needed inputs (A_0, all four Wj pairs) ride the two fast HWDGE
  queues; the remaining anchors ride the gpsimd SWDGE queue in parallel
  with the store stream. Tile 0 is stored in four 512 KB sub-stores so
  the HBM write stream starts as soon as the first copy lands.
- b is folded in on the host only if nonzero (it is zero in this
  problem's setup_inputs); the device path is a pure GEMM.
"""

import sys

if "/opt/trn_rl_repo" not in sys.path:
    sys.path.insert(0, "/opt/trn_rl_repo")

import numpy as np

L = 131072
S = 128  # num states (= partition dim = contraction dim)
E = 512  # embed dim
NCORES = 8
CHUNK = L // NCORES  # 16384 rows per core
K = 8  # rows per partition per store tile (decimation factor)
TILE_ROWS = S * K  # 1024 rows per store tile
NTILES = CHUNK // TILE_ROWS  # 16 store tiles per core
NPAIR = K // 2  # 4 psum pairs per tile
WARMUP = 20  # dummy matmuls to trip the HAM clock gate

_prog_cache = {}


def _split_multi_waits(nc, mybir):
    """This walrus build accepts only ONE sync-wait per instruction
    (setupSyncWait: 'Too many sync wait commands'). Tile attaches the
    full wait list to the consuming instruction; hoist all but the
    last wait onto single-wait NoOps placed immediately before it on
    the same engine, preserving per-engine program order."""
    uid = 0
    for fn in nc.m.functions:
        for bb in fn.blocks:
            new = []
            changed = False
            for inst in bb.instructions:
                si = inst.sync_info
                waits = list(si.on_wait) if si is not None else []
                if len(waits) > 1:
                    changed = True
                    for w in waits[:-1]:
                        nop = mybir.InstNoOp(
                            name=f"splitw_{uid}",
                            engine=inst.engine,
                            sync_info=mybir.SyncInfo(on_wait=[w], on_update=[]),
                            bass_nofuse=True,
                        )
                        uid += 1
                        new.append(nop)
                    si.on_wait = [waits[-1]]
                new.append(inst)
            if changed:
                bb.instructions = new


def _build_program():
    if "nc" in _prog_cache:
        return _prog_cache["nc"]

    import concourse.bass as bass
    import concourse.tile as tile
    from concourse import mybir

    f32 = mybir.dt.float32
    f16 = mybir.dt.float16
    nc = bass.Bass("TRN2", target_bir_lowering=False, debug=False, num_devices=NCORES)

    # anchors differ per core; wpairs replicated. fp16 operands: single-pass
    # PE matmul (1 cycle/column) with fast weight load; fp32 PSUM accumulate.
    anchors = nc.dram_tensor("anchors", [NTILES, S, S], f16, kind="ExternalInput").ap()
    wpairs = nc.dram_tensor("wpairs", [NPAIR, S, 2 * E], f16, kind="ExternalInput").ap()
    out = nc.dram_tensor("out", [CHUNK, E], f32, kind="ExternalOutput").ap()

    anchors_v = anchors.rearrange("t s p -> s t p")  # [S, NTILES, S]
    wpairs_v = wpairs.rearrange("q s e -> s q e")  # [S, NPAIR, 2E]

    # Row base+8p+j of the output lives at DRAM row t*1024 + p*8 + j:
    # full-tile view [t, p, (k e)] is 16 KB contiguous per partition.
    out_t = out.rearrange("(t p k) e -> t p (k e)", p=S, k=K)  # [NTILES, S, K*E]
    # Sub-store view per psum pair q (rows j=2q, 2q+1): 4 KB per partition.
    out_q = out.rearrange("(t p q j) e -> t q p (j e)", p=S, q=NPAIR, j=2)

    with tile.TileContext(nc) as tc:
        with (
            tc.tile_pool(name="singles", bufs=1) as singles,
            tc.tile_pool(name="opool", bufs=5) as opool,
            tc.tile_pool(name="psum", bufs=3, space="PSUM") as psum,
            tc.tile_pool(name="warm", bufs=1, space="PSUM") as warm,
        ):
            anch_t = singles.tile([S, NTILES, S], f16)
            wp_t = singles.tile([S, NPAIR, 2 * E], f16)
            dummy = singles.tile([S, S], f16)

            # PE warmup: sustained dummy matmuls (no input deps) flip the
            # HAM clock gate to 2.4 GHz while the inputs load.
            nc.vector.memset(dummy, 0.0)

            # First-needed inputs on the two HWDGE rings, in need order:
            # sync ring: W(0|1), W(4|5); scalar ring: A_0, W(2|3), A_1, W(6|7).
            nc.sync.dma_start(out=wp_t[:, 0, :], in_=wpairs_v[:, 0, :])
            nc.scalar.dma_start(out=anch_t[:, 0, :], in_=anchors_v[:, 0, :])
            nc.scalar.dma_start(out=wp_t[:, 1, :], in_=wpairs_v[:, 1, :])
            nc.sync.dma_start(out=wp_t[:, 2, :], in_=wpairs_v[:, 2, :])
            nc.scalar.dma_start(out=anch_t[:, 1, :], in_=anchors_v[:, 1, :])
            nc.scalar.dma_start(out=wp_t[:, 3, :], in_=wpairs_v[:, 3, :])
            # Remaining anchors individually on the HWDGE rings, ahead of
            # the stores in ring order (32 KB each, ~0.1 us). Avoiding the
            # gpsimd SWDGE queue keeps its descriptor rings silent: SWDGE
            # ring traffic starves SDMA engine 15, which then lags the
            # whole store stream (observed +12 us on e15's packets).
            for t in range(2, NTILES):
                eng = nc.sync if t % 2 == 0 else nc.scalar
                eng.dma_start(out=anch_t[:, t, :], in_=anchors_v[:, t, :])

            for _ in range(WARMUP):
                pw = warm.tile([S, E], f32)
                nc.tensor.matmul(pw[:, :S], dummy, dummy, start=True, stop=True)

            for t in range(NTILES):
                o_t = opool.tile([S, K * E], f32)
                for q in range(NPAIR):
                    ps = psum.tile([S, 2 * E], f32)
                    for j in range(2):
                        nc.tensor.matmul(
                            ps[:, j * E : (j + 1) * E],
                            anch_t[:, t, :],
                            wp_t[:, q, j * E : (j + 1) * E],
                            start=True,
                            stop=True,
                        )
                    o_sl = o_t[:, q * 2 * E : (q + 1) * 2 * E]
                    if q % 2 == 0:
                        nc.vector.tensor_copy(o_sl, ps)
                    else:
                        nc.scalar.copy(out=o_sl, in_=ps)
                    if t == 0:
                        # Head: stream each 512 KB quarter as soon as its
                        # copy lands.
                        if q % 2 == 0:
                            nc.sync.dma_start(out=out_q[0, q], in_=o_sl)
                        else:
                            nc.scalar.dma_start(out=out_q[0, q], in_=o_sl)
                if t == NTILES - 1:
                    # Tail: drain the last tile as two 1 MB halves on both
                    # rings so the final store finishes sooner.
                    half = K * E // 2
                    nc.sync.dma_start(
                        out=out_t[t][:, :half], in_=o_t[:, :half]
                    )
                    nc.scalar.dma_start(
                        out=out_t[t][:, half:], in_=o_t[:, half:]
                    )
                elif t > 0:
                    if t % 2 == 0:
                        nc.sync.dma_start(out=out_t[t], in_=o_t)
                    else:
                        nc.scalar.dma_start(out=out_t[t], in_=o_t)

    _split_multi_waits(nc, mybir)
    _prog_cache["nc"] = nc
    return nc


def _host_precompute(pos_initial, pos_transition, W):
    """float64 host prep: decimated per-core anchors + T^j-folded weights."""
    T = np.asarray(pos_transition, np.float64)
    x0 = np.asarray(pos_initial, np.float64).reshape(S)
    W64 = np.asarray(W, np.float64)

    # T^8 and T^1024 by repeated squaring
    T8 = T.copy()
    for _ in range(3):
        T8 = T8 @ T8
    T1024 = T8.copy()
    for _ in range(7):
        T1024 = T1024 @ T1024

    # Wj = (T^j)^T @ W^T for j = 0..K-1, paired as [NPAIR, S, 2E]
    Tj = np.eye(S)
    wjs = []
    for j in range(K):
        wjs.append(np.ascontiguousarray(Tj.T @ W64.T))
        Tj = T @ Tj
    wpairs = np.stack(
        [np.concatenate([wjs[2 * q], wjs[2 * q + 1]], axis=1) for q in range(NPAIR)]
    ).astype(np.float16)

    # Anchor columns: y_i = T^(8 i) x0. A(m, t)[:, p] = y[(m*16 + t)*128 + p].
    C = np.empty((S, S), np.float64)
    v = x0.copy()
    C[:, 0] = v
    for i in range(1, S):
        v = T8 @ v
        C[:, i] = v
    anchor_steps = []
    A = C
    for _ in range(NCORES * NTILES):
        anchor_steps.append(A)
        A = T1024 @ A
    anchors_all = np.asarray(anchor_steps, np.float64).reshape(NCORES, NTILES, S, S)
    anchors = [
        np.ascontiguousarray(anchors_all[m]).astype(np.float16) for m in range(NCORES)
    ]
    return anchors, wpairs


def kernel(sentence_len, pos_initial, pos_transition, W, b):
    from concourse.bass_utils import run_bass_kernel_spmd

    assert int(sentence_len) == L, f"kernel hardcodes L={L}, got {sentence_len}"
    b = np.asarray(b, np.float32)

    anchors, wpairs = _host_precompute(pos_initial, pos_transition, W)

    nc = _build_program()
    in_maps = [{"anchors": anchors[m], "wpairs": wpairs} for m in range(NCORES)]
    res = run_bass_kernel_spmd(nc, in_maps, core_ids=list(range(NCORES)))
    full = np.concatenate([res.results[m]["out"] for m in range(NCORES)], axis=0)
    if np.any(b != 0):
        full = full + b[None, :]
    return full
